# revision 15
# baseline (speedup 1.0000x reference)
"""MultiHeadAttention forward on 8 Trainium2 NeuronCores.

Problem: B=2, S=2048, D_MODEL=1024, H=16 heads, d_k=64, causal mask.

Sharding: core c in [0,8) owns heads {2c, 2c+1} for BOTH batches.
 - Projections: each core computes Q^T,K^T ([128=2*d_k, B*S]) and V
   ([B*S, 2*65]) for its 2 heads (contraction over full d_model).
 - Attention in the "transposed scores" orientation: scoresT[kpos, q] =
   K^T.T @ Q^T per head (the two heads' 64-contraction matmuls run
   row-tiled/concurrently on the PE), exp on ScalarE (scale=1/8 folded
   in), causal handling is trimmed to the live column range of each
   diagonal tile plus a single [128,128] triangular mask multiply.
   attn_outT[dv, q] accumulates via matmuls with V tiles stationary; a
   ones-column in V yields softmax denominators in the same matmul.
 - Normalization: reciprocal_approx_fast of the sums row, gpsimd
   partition-broadcast, multiply into bf16 attn_flatT [128, B*S].
 - Output: NO collective. Each core multiplies its 128 attn_flatT rows
   by its 128 rows of W_o, producing a PARTIAL [B*S, D] output in bf16;
   the host sums the 8 partials and adds b_o (the all-reduce is the
   host-side unshard). W_o chunks + output stores pipeline per q-block,
   so no serial tail remains.

Matmuls run in bf16 (host-cast); accumulation is fp32 in PSUM.
"""

import sys

import numpy as np

sys.path.insert(0, "/opt/trn_rl_repo")

import ml_dtypes  # noqa: E402

import concourse.bacc as bacc  # noqa: E402
import concourse.mybir as mybir  # noqa: E402
import concourse.tile as tile  # noqa: E402
from concourse.bass_utils import run_bass_kernel_spmd  # noqa: E402

F32 = mybir.dt.float32
BF16 = mybir.dt.bfloat16
BF = ml_dtypes.bfloat16

B, S, D, H, DK = 2, 2048, 1024, 16, 64
N_CORES = 8
BS = B * S  # 4096
HPC = H // N_CORES  # heads per core = 2
DPC = HPC * DK  # d_model slice per core = 128

_CACHED = {}


class _Bacc(bacc.Bacc):
    """Bacc that pins the ACT function table to natural_log_exp_and_others.

    The default table pass picks the FIRST act_info.json set containing each
    activation: exp -> set 0, ln -> set 5, which makes every softmax-denominator
    reciprocal (ln then exp) swap the ACT table twice (~1.3us each) and
    serialize the exp queue. Removing exp/ln from every other set (list order
    and hence act_func_set ids unchanged) forces the one set that has both.
    """

    def insert_act_table_loads(self):
        import bass_rust as _bass_rust
        from concourse.hw_specs import get_activation_tables

        has_activation = any(
            isinstance(i, mybir.InstActivation)
            for blk in self.main_func.blocks
            for i in blk.instructions
        )
        if not has_activation:
            return
        A = mybir.ActivationFunctionType
        both = "natural_log_exp_and_others"
        tables = []
        for name, fns in get_activation_tables(self.m.arch).items():
            if name != both:
                fns = fns - {A.Exp, A.Ln}
            tables.append((name, fns))
        assert any(name == both and A.Exp in fns and A.Ln in fns for name, fns in tables)
        _bass_rust.insert_act_table_loads(self, tables)


def build_nc():
    nc = _Bacc(num_devices=N_CORES)

    # ---- I/O ----
    # x layout: [p, (b, h, j, c)] where element = x^T[128j+p, 2048b+1024h+c]
    xq = nc.dram_tensor("xq", [128, B * 2 * 8 * 1024], BF16, kind="ExternalInput")
    xk = nc.dram_tensor("xk", [128, B * 2 * 8 * 1024], BF16, kind="ExternalInput")
    xv = nc.dram_tensor("xv", [128, B * 2 * 8 * 1024], BF16, kind="ExternalInput")
    # w layout: [p, (j, c)] = W[128j+p, my_cols c]
    wq = nc.dram_tensor("wq", [128, 1024], BF16, kind="ExternalInput")
    wk = nc.dram_tensor("wk", [128, 1024], BF16, kind="ExternalInput")
    wv = nc.dram_tensor("wv", [128, 1024], BF16, kind="ExternalInput")
    wo = nc.dram_tensor("wo", [128, D], BF16, kind="ExternalInput")  # my 128 rows
    bq = nc.dram_tensor("bq", [DPC, 1], F32, kind="ExternalInput")
    bk = nc.dram_tensor("bk", [DPC, 1], F32, kind="ExternalInput")
    bv = nc.dram_tensor("bv", [1, DPC], F32, kind="ExternalInput")
    mask = nc.dram_tensor("mask", [128, 128], BF16, kind="ExternalInput")
    # out layout: [p, ((b*4+qb)*4 + ss)*1024 + c] = partial[2048b+512qb+128ss+p, c]
    out = nc.dram_tensor("out", [128, 32 * 1024], BF16, kind="ExternalOutput")

    NKT = S // 128  # kpos tiles per batch = 16
    NQB = S // 512  # q blocks per batch = 4

    with tile.TileContext(nc) as tc:
        with (
            tc.tile_pool(name="xtq", bufs=5) as xq_pool,
            tc.tile_pool(name="xtk", bufs=5) as xk_pool,
            tc.tile_pool(name="xtv", bufs=4) as xv_pool,
            tc.tile_pool(name="wtiles", bufs=1) as w_pool,
            tc.tile_pool(name="persist", bufs=1) as persist,
            tc.tile_pool(name="exp", bufs=4) as exp_pool,
            tc.tile_pool(name="outsb", bufs=2) as out_pool,
            tc.tile_pool(name="small", bufs=2) as small_pool,
            tc.tile_pool(name="gen_ps", bufs=2, space="PSUM") as gen_ps,
            tc.tile_pool(name="score_ps", bufs=2, space="PSUM") as score_ps,
            tc.tile_pool(name="av_ps", bufs=1, space="PSUM") as av_ps,
        ):
            # ---- persistent SBUF tensors ----
            QT = persist.tile([128, BS], BF16, tag="QT")  # rows: hA d 0-63, hB 64-127
            KT = persist.tile([128, BS], BF16, tag="KT")
            VA = [persist.tile([128, DK + 1], BF16, tag=f"VA{i}", name=f"VA{i}") for i in range(2 * NKT)]
            VB = [persist.tile([128, DK + 1], BF16, tag=f"VB{i}", name=f"VB{i}") for i in range(2 * NKT)]
            AFT = persist.tile([128, BS], BF16, tag="AFT")  # attn_flatT

            # ---- small loads first (tiny), then x chunks in use-order ----
            wq_t = persist.tile([128, 1024], BF16, tag="wqt")
            wk_t = persist.tile([128, 1024], BF16, tag="wkt")
            wv_t = persist.tile([128, 1024], BF16, tag="wvt")
            wo_t = persist.tile([128, D], BF16, tag="wot")
            # x chunk tiles, loaded per (batch, 512-token chunk): [128, 4096]
            def load_x(pool, src, b, h):
                t = pool.tile([128, 4096], BF16, tag="xc", name="xc")
                off = (b * 4 + h) * 4096
                nc.sync.dma_start(t[:], src[:, off : off + 4096])
                return t

            xq_t = [[None] * 4 for _ in range(B)]
            xk_t = [[None] * 4 for _ in range(B)]
            xv_t = [[None] * 4 for _ in range(B)]
            # first chunk triplet first: it gates the first projection chains
            xq_t[0][0] = load_x(xq_pool, xq, 0, 0)
            nc.sync.dma_start(wq_t[:], wq[:])
            nc.sync.dma_start(wk_t[:], wk[:])
            xk_t[0][0] = load_x(xk_pool, xk, 0, 0)
            nc.sync.dma_start(wv_t[:], wv[:])
            xv_t[0][0] = load_x(xv_pool, xv, 0, 0)
            bq_t = persist.tile([DPC, 1], F32, tag="bq")
            bk_t = persist.tile([DPC, 1], F32, tag="bk")
            nc.sync.dma_start(bq_t[:], bq[:])
            nc.sync.dma_start(bk_t[:], bk[:])
            bv_bc = persist.tile([128, DPC], F32, tag="bvbc")
            nc.sync.dma_start(bv_bc[:], bv[:].partition_broadcast(128))
            mask_t = persist.tile([128, 128], BF16, tag="mask")
            nc.sync.dma_start(mask_t[:], mask[:])
            for b in range(B):
                for h in range(4):
                    if b == 0 and h == 0:
                        continue
                    xq_t[b][h] = load_x(xq_pool, xq, b, h)
                    xk_t[b][h] = load_x(xk_pool, xk, b, h)
                    xv_t[b][h] = load_x(xv_pool, xv, b, h)
                if b == 0:
                    nc.sync.dma_start(wo_t[:], wo[:])

            # ---- unit emitters ----
            def proj_chain_qk(b, sc, which):
                """one psum chain: Q or K projection for 512-token chunk sc"""
                scol = S * b
                xt_ = xq_t[b] if which == "q" else xk_t[b]
                wt_ = wq_t if which == "q" else wk_t
                bias = bq_t if which == "q" else bk_t
                dst = QT if which == "q" else KT
                ps = gen_ps.tile([128, 512], F32, tag="gen", name="gps")
                for j in range(8):
                    nc.tensor.matmul(
                        ps[:],
                        wt_[:, 128 * j : 128 * (j + 1)],
                        xt_[sc][:, 512 * j : 512 * (j + 1)],
                        start=(j == 0),
                        stop=(j == 7),
                    )
                nc.vector.tensor_scalar_add(
                    dst[:, scol + 512 * sc : scol + 512 * (sc + 1)], ps[:], bias[:]
                )

            def proj_chain_v(b, ss):
                """one psum chain: V projection for 128-token block ss"""
                h, jj = ss // 4, ss % 4
                ps = gen_ps.tile([128, 128], F32, tag="gen", name="gps")
                for j in range(8):
                    nc.tensor.matmul(
                        ps[:],
                        xv_t[b][h][:, 512 * j + 128 * jj : 512 * j + 128 * (jj + 1)],
                        wv_t[:, 128 * j : 128 * (j + 1)],
                        start=(j == 0),
                        stop=(j == 7),
                    )
                va = VA[NKT * b + ss]
                vb = VB[NKT * b + ss]
                nc.vector.tensor_add(va[:, 0:DK], ps[:, 0:DK], bv_bc[:, 0:DK])
                nc.vector.tensor_add(vb[:, 0:DK], ps[:, DK : 2 * DK], bv_bc[:, DK : 2 * DK])
                nc.vector.memset(va[:, DK : DK + 1], 1.0)
                nc.vector.memset(vb[:, DK : DK + 1], 1.0)

            def proj_unit_fillers(b, c):
                """chunk c of batch b -> 6 filler chains"""
                return [
                    lambda b=b, c=c: proj_chain_qk(b, c, "q"),
                    lambda b=b, c=c: proj_chain_qk(b, c, "k"),
                    lambda b=b, c=c: proj_chain_v(b, 4 * c + 0),
                    lambda b=b, c=c: proj_chain_v(b, 4 * c + 1),
                    lambda b=b, c=c: proj_chain_v(b, 4 * c + 2),
                    lambda b=b, c=c: proj_chain_v(b, 4 * c + 3),
                ]

            osb_t = {}

            def wo_chain(b, qb, ssub):
                """partial W_o for one 128-token slice (host sums partials)"""
                qsl0 = S * b + 512 * qb
                asl = slice(qsl0 + 128 * ssub, qsl0 + 128 * (ssub + 1))
                if ssub == 0:
                    osb_t[(b, qb)] = out_pool.tile([128, 4096], BF16, tag="osb", name="osb")
                osb = osb_t[(b, qb)]
                for nch in range(2):
                    ps = gen_ps.tile([128, 512], F32, tag="gen", name="gps")
                    nc.tensor.matmul(
                        ps[:],
                        AFT[:, asl],
                        wo_t[:, 512 * nch : 512 * (nch + 1)],
                        start=True,
                        stop=True,
                    )
                    nc.vector.tensor_copy(
                        osb[:, 1024 * ssub + 512 * nch : 1024 * ssub + 512 * (nch + 1)],
                        ps[:],
                    )
                ocol = (b * 4 + qb) * 4096
                if (b, qb) == (1, 3):
                    # final unit: store each 128-token slice as it completes
                    nc.gpsimd.dma_start(
                        out[:, ocol + 1024 * ssub : ocol + 1024 * (ssub + 1)],
                        osb[:, 1024 * ssub : 1024 * (ssub + 1)],
                    )
                elif ssub == 3:
                    nc.gpsimd.dma_start(out[:, ocol : ocol + 4096], osb[:])

            def wo_fillers(b, qb):
                return [lambda s=s: wo_chain(b, qb, s) for s in range(4)]

            avs_t = {}

            def attn_unit(b, qb, fillers):
                """scores/exp/mask/attnV for one q-block; fillers are emitted
                between kt iterations to keep the in-order PE queue fed
                during the exp-bound stretches"""
                scol = S * b
                qsl0 = scol + 512 * qb
                n_kt = 4 * qb + 4

                def emit_scores(kt):
                    t = kt - 4 * qb
                    lo = 128 * t if t >= 0 else 0
                    ksl = slice(scol + 128 * kt, scol + 128 * (kt + 1))
                    qsl = slice(qsl0 + lo, qsl0 + 512)
                    ps = score_ps.tile([128, 1024], F32, tag="sc", name="sc")
                    nc.tensor.matmul(
                        ps[:, lo:512], KT[0:64, ksl], QT[0:64, qsl],
                        start=True, stop=True,
                    )
                    nc.tensor.matmul(
                        ps[:, 512 + lo : 1024], KT[64:128, ksl], QT[64:128, qsl],
                        start=True, stop=True,
                    )
                    return ps

                av_a = av_ps.tile([DK + 1, 512], F32, tag="av_a", name="av_a")
                av_b = av_ps.tile([DK + 1, 512], F32, tag="av_b", name="av_b")
                fq = list(fillers)
                ps_cur = emit_scores(0)
                # a couple of fillers right away: PE-queue work while the
                # first exp waits behind any fin ops on the ACT queue
                for _ in range(min(2, len(fq))):
                    fq.pop(0)()
                # spread the rest roughly evenly over the kt loop
                nf = len(fq)
                emit_at = set()
                if nf:
                    for i in range(nf):
                        emit_at.add(1 + (i * max(1, (n_kt - 1))) // nf)
                for kt in range(n_kt):
                    t = kt - 4 * qb
                    lo = 128 * t if t >= 0 else 0
                    et = exp_pool.tile([128, 1024], BF16, tag="et", name="et")
                    if t >= 0:
                        et3 = et[:].rearrange("p (h c) -> p h c", h=2)
                        ps3 = ps_cur[:].rearrange("p (h c) -> p h c", h=2)
                        nc.scalar.activation(
                            et3[:, :, lo:512], ps3[:, :, lo:512],
                            mybir.ActivationFunctionType.Exp, scale=0.125,
                        )
                    else:
                        nc.scalar.activation(
                            et[:], ps_cur[:], mybir.ActivationFunctionType.Exp,
                            scale=0.125,
                        )
                    if kt + 1 < n_kt:
                        ps_cur = emit_scores(kt + 1)
                    if t >= 0:
                        et3 = et[:].rearrange("p (h c) -> p h c", h=2)
                        nc.vector.tensor_mul(
                            et3[:, :, lo : lo + 128],
                            et3[:, :, lo : lo + 128],
                            mask_t[:].unsqueeze(1).broadcast_to((128, 2, 128)),
                        )
                    nc.tensor.matmul(
                        av_a[:, lo:512], VA[NKT * b + kt][:], et[:, lo:512],
                        start=(kt == 0), stop=(kt == n_kt - 1),
                    )
                    nc.tensor.matmul(
                        av_b[:, lo:512], VB[NKT * b + kt][:], et[:, 512 + lo : 1024],
                        start=(kt == 0), stop=(kt == n_kt - 1),
                    )
                    if kt in emit_at:
                        while fq and kt in emit_at:
                            fq.pop(0)()
                            break
                # evacuate psum promptly (frees attnV banks for next q-block)
                avs = small_pool.tile([DK + 1, 1024], F32, tag="avs", name="avs")
                nc.vector.tensor_copy(avs[:, 0:512], av_a[:])
                nc.vector.tensor_copy(avs[:, 512:1024], av_b[:])
                avs_t[(b, qb)] = avs
                for f in fq:
                    f()

            def fin_unit(b, qb):
                """normalize q-block qb into AFT (delayed one unit so the
                ACT-side reciprocal never blocks the exp queue)"""
                qsl = slice(S * b + 512 * qb, S * b + 512 * (qb + 1))
                avs = avs_t.pop((b, qb))
                # 1/den as exp(-ln(den)) on ScalarE, both heads in one pass
                lnd = small_pool.tile([1, 1024], F32, tag="scr", name="lnd")
                nc.scalar.activation(
                    lnd[:], avs[DK : DK + 1, :],
                    mybir.ActivationFunctionType.Ln,
                )
                rc = small_pool.tile([1, 1024], F32, tag="scr", name="rc")
                nc.scalar.activation(
                    rc[:], lnd[:], mybir.ActivationFunctionType.Exp,
                    scale=-1.0,
                )
                rbc = small_pool.tile([64, 1024], F32, tag="rbc", name="rbc")
                nc.gpsimd.partition_broadcast(rbc[:], rc[:])
                for hh in range(2):
                    nc.vector.tensor_mul(
                        AFT[64 * hh : 64 * (hh + 1), qsl],
                        avs[0:DK, 512 * hh : 512 * (hh + 1)],
                        rbc[:, 512 * hh : 512 * (hh + 1)],
                    )

            # ---- emission schedule (in-order engine queues: program order
            # is execution order, so software-pipeline across units) ----
            for f in proj_unit_fillers(0, 0):
                f()
            for f in proj_unit_fillers(0, 1):
                f()
            attn_unit(0, 0, [])
            for f in proj_unit_fillers(0, 2):
                f()
            fin_unit(0, 0)
            attn_unit(0, 1, wo_fillers(0, 0))
            for f in proj_unit_fillers(0, 3):
                f()
            fin_unit(0, 1)
            attn_unit(0, 2, wo_fillers(0, 1))
            fin_unit(0, 2)
            attn_unit(0, 3, proj_unit_fillers(1, 0) + wo_fillers(0, 2))
            attn_unit(1, 0, proj_unit_fillers(1, 1)[:4])
            fin_unit(0, 3)
            for f in proj_unit_fillers(1, 1)[4:]:
                f()
            fin_unit(1, 0)
            attn_unit(1, 1, wo_fillers(0, 3) + wo_fillers(1, 0))
            for f in proj_unit_fillers(1, 2):
                f()
            fin_unit(1, 1)
            attn_unit(1, 2, proj_unit_fillers(1, 3))
            fin_unit(1, 2)
            attn_unit(1, 3, wo_fillers(1, 1) + wo_fillers(1, 2))
            fin_unit(1, 3)
            for f in wo_fillers(1, 3):
                f()

    nc.finalize()
    return nc


def _prep_in_maps(q, k, v, W_q, b_q, W_k, b_k, W_v, b_v, W_o, b_o):
    def xT(x):  # [B,S,D] f32 -> [128, B*2*8*1024] bf16, chunked layout
        t = x.reshape(BS, D).T.astype(BF)  # [D, BS]
        # [j, p, b, sc, c] -> [p, b, sc, j, c]
        t = t.reshape(8, 128, B, 4, 512).transpose(1, 2, 3, 0, 4)
        return np.ascontiguousarray(t.reshape(128, B * 2 * 8 * 1024))

    def wpack(w):  # [D, 128] -> [128, 8*128]
        t = w.astype(BF).reshape(8, 128, DPC).transpose(1, 0, 2)
        return np.ascontiguousarray(t.reshape(128, 8 * DPC))

    xq_h, xk_h, xv_h = xT(q), xT(k), xT(v)

    i = np.arange(128)[:, None]
    j = np.arange(128)[None, :]
    mask_h = np.ascontiguousarray((i <= j).astype(BF))

    in_maps = []
    for c in range(N_CORES):
        csl = slice(DPC * c, DPC * (c + 1))
        in_maps.append(
            {
                "xq": xq_h,
                "xk": xk_h,
                "xv": xv_h,
                "wq": wpack(W_q[:, csl]),
                "wk": wpack(W_k[:, csl]),
                "wv": wpack(W_v[:, csl]),
                "wo": np.ascontiguousarray(W_o[csl, :].astype(BF)),
                "bq": np.ascontiguousarray(
                    b_q[csl].reshape(DPC, 1).astype(np.float32)
                ),
                "bk": np.ascontiguousarray(
                    b_k[csl].reshape(DPC, 1).astype(np.float32)
                ),
                "bv": np.ascontiguousarray(
                    b_v[csl].reshape(1, DPC).astype(np.float32)
                ),
                "mask": mask_h,
            }
        )
    return in_maps


def kernel(q, k, v, mask, W_q, b_q, W_k, b_k, W_v, b_v, W_o, b_o, **run_kwargs):
    q, k, v = (np.asarray(t, np.float32) for t in (q, k, v))
    b_o = np.asarray(b_o, np.float32)
    in_maps = _prep_in_maps(
        q, k, v,
        np.asarray(W_q, np.float32), np.asarray(b_q, np.float32),
        np.asarray(W_k, np.float32), np.asarray(b_k, np.float32),
        np.asarray(W_v, np.float32), np.asarray(b_v, np.float32),
        np.asarray(W_o, np.float32), b_o,
    )
    if "nc" not in _CACHED:
        _CACHED["nc"] = build_nc()
    res = run_bass_kernel_spmd(
        _CACHED["nc"], in_maps, core_ids=list(range(N_CORES)), **run_kwargs
    )
    _CACHED["last_result"] = res
    # host-side all-reduce of the 8 partial outputs (the unshard step)
    acc = None
    for c in range(N_CORES):
        part = np.asarray(res.results[c]["out"])  # [128, 32*1024] bf16
        part = part.reshape(128, B, 4, 4, 1024).transpose(1, 2, 3, 0, 4)
        part = part.reshape(BS, D).astype(np.float32)
        acc = part if acc is None else acc + part
    acc += b_o.reshape(1, D)
    return acc.reshape(B, S, D)


if __name__ == "__main__":
    build_nc()
    print("build ok")


# revision 16
# speedup vs baseline: 1.1067x; 1.1067x over previous
"""MultiHeadAttention forward on 8 Trainium2 NeuronCores.

Problem: B=2, S=2048, D_MODEL=1024, H=16 heads, d_k=64, causal mask.

Sharding: core c in [0,8) owns heads {2c, 2c+1} for BOTH batches.
 - Projections: each core computes Q^T,K^T ([128=2*d_k, B*S]) and V
   ([B*S, 2*65]) for its 2 heads (contraction over full d_model).
 - Attention in the "transposed scores" orientation: scoresT[kpos, q] =
   K^T.T @ Q^T per head (the two heads' 64-contraction matmuls run
   row-tiled/concurrently on the PE), exp on ScalarE (scale=1/8 folded
   in), causal handling is trimmed to the live column range of each
   diagonal tile plus a single [128,128] triangular mask multiply.
   attn_outT[dv, q] accumulates via matmuls with V tiles stationary; a
   ones-column in V yields softmax denominators in the same matmul.
 - Normalization: reciprocal_approx_fast of the sums row, gpsimd
   partition-broadcast, multiply into bf16 attn_flatT [128, B*S].
 - Output: NO collective. Each core multiplies its 128 attn_flatT rows
   by its 128 rows of W_o, producing a PARTIAL [B*S, D] output in bf16;
   the host sums the 8 partials and adds b_o (the all-reduce is the
   host-side unshard). W_o chunks + output stores pipeline per q-block,
   so no serial tail remains.

Matmuls run in bf16 (host-cast); accumulation is fp32 in PSUM.
"""

import sys

import numpy as np

sys.path.insert(0, "/opt/trn_rl_repo")

import ml_dtypes  # noqa: E402

import concourse.bacc as bacc  # noqa: E402
import concourse.mybir as mybir  # noqa: E402
import concourse.tile as tile  # noqa: E402
from concourse.bass_utils import run_bass_kernel_spmd  # noqa: E402

F32 = mybir.dt.float32
BF16 = mybir.dt.bfloat16
BF = ml_dtypes.bfloat16

B, S, D, H, DK = 2, 2048, 1024, 16, 64
N_CORES = 8
BS = B * S  # 4096
HPC = H // N_CORES  # heads per core = 2
DPC = HPC * DK  # d_model slice per core = 128

_CACHED = {}


class _Bacc(bacc.Bacc):
    """Bacc that pins the ACT function table to natural_log_exp_and_others.

    The default table pass picks the FIRST act_info.json set containing each
    activation: exp -> set 0, ln -> set 5, which makes every softmax-denominator
    reciprocal (ln then exp) swap the ACT table twice (~1.3us each) and
    serialize the exp queue. Removing exp/ln from every other set (list order
    and hence act_func_set ids unchanged) forces the one set that has both.
    """

    def insert_act_table_loads(self):
        import bass_rust as _bass_rust
        from concourse.hw_specs import get_activation_tables

        has_activation = any(
            isinstance(i, mybir.InstActivation)
            for blk in self.main_func.blocks
            for i in blk.instructions
        )
        if not has_activation:
            return
        A = mybir.ActivationFunctionType
        both = "natural_log_exp_and_others"
        tables = []
        for name, fns in get_activation_tables(self.m.arch).items():
            if name != both:
                fns = fns - {A.Exp, A.Ln}
            tables.append((name, fns))
        assert any(name == both and A.Exp in fns and A.Ln in fns for name, fns in tables)
        _bass_rust.insert_act_table_loads(self, tables)


def build_nc():
    nc = _Bacc(num_devices=N_CORES)

    # ---- I/O ----
    # x layout: [p, (b, h, j, c)] where element = x^T[128j+p, 2048b+1024h+c]
    xq = nc.dram_tensor("xq", [128, B * 2 * 8 * 1024], BF16, kind="ExternalInput")
    xk = nc.dram_tensor("xk", [128, B * 2 * 8 * 1024], BF16, kind="ExternalInput")
    xv = nc.dram_tensor("xv", [128, B * 2 * 8 * 1024], BF16, kind="ExternalInput")
    # w layout: [p, (j, c)] = W[128j+p, my_cols c]
    wq = nc.dram_tensor("wq", [128, 1024], BF16, kind="ExternalInput")
    wk = nc.dram_tensor("wk", [128, 1024], BF16, kind="ExternalInput")
    wv = nc.dram_tensor("wv", [128, 1024], BF16, kind="ExternalInput")
    wo = nc.dram_tensor("wo", [128, D], BF16, kind="ExternalInput")  # my 128 rows
    bq = nc.dram_tensor("bq", [DPC, 1], F32, kind="ExternalInput")
    bk = nc.dram_tensor("bk", [DPC, 1], F32, kind="ExternalInput")
    bv = nc.dram_tensor("bv", [1, DPC], F32, kind="ExternalInput")
    mask = nc.dram_tensor("mask", [128, 128], BF16, kind="ExternalInput")
    # out layout: [p, ((b*4+qb)*4 + ss)*1024 + c] = partial[2048b+512qb+128ss+p, c]
    out = nc.dram_tensor("out", [128, 32 * 1024], BF16, kind="ExternalOutput")

    NKT = S // 128  # kpos tiles per batch = 16
    NQB = S // 512  # q blocks per batch = 4

    with tile.TileContext(nc) as tc:
        with (
            tc.tile_pool(name="xtq", bufs=5) as xq_pool,
            tc.tile_pool(name="xtk", bufs=5) as xk_pool,
            tc.tile_pool(name="xtv", bufs=4) as xv_pool,
            tc.tile_pool(name="wtiles", bufs=1) as w_pool,
            tc.tile_pool(name="persist", bufs=1) as persist,
            tc.tile_pool(name="exp", bufs=4) as exp_pool,
            tc.tile_pool(name="outsb", bufs=2) as out_pool,
            tc.tile_pool(name="small", bufs=2) as small_pool,
            tc.tile_pool(name="gen_ps", bufs=2, space="PSUM") as gen_ps,
            tc.tile_pool(name="score_ps", bufs=2, space="PSUM") as score_ps,
            tc.tile_pool(name="av_ps", bufs=1, space="PSUM") as av_ps,
        ):
            # ---- persistent SBUF tensors ----
            QT = persist.tile([128, BS], BF16, tag="QT")  # rows: hA d 0-63, hB 64-127
            KT = persist.tile([128, BS], BF16, tag="KT")
            VA = [persist.tile([128, DK + 1], BF16, tag=f"VA{i}", name=f"VA{i}") for i in range(2 * NKT)]
            VB = [persist.tile([128, DK + 1], BF16, tag=f"VB{i}", name=f"VB{i}") for i in range(2 * NKT)]
            AFT = persist.tile([128, BS], BF16, tag="AFT")  # attn_flatT

            # ---- small loads first (tiny), then x chunks in use-order ----
            wq_t = persist.tile([128, 1024], BF16, tag="wqt")
            wk_t = persist.tile([128, 1024], BF16, tag="wkt")
            wv_t = persist.tile([128, 1024], BF16, tag="wvt")
            wo_t = persist.tile([128, D], BF16, tag="wot")
            # x chunk tiles, loaded per (batch, 512-token chunk): [128, 4096]
            def load_x(pool, src, b, h):
                t = pool.tile([128, 4096], BF16, tag="xc", name="xc")
                off = (b * 4 + h) * 4096
                nc.sync.dma_start(t[:], src[:, off : off + 4096])
                return t

            xq_t = [[None] * 4 for _ in range(B)]
            xk_t = [[None] * 4 for _ in range(B)]
            xv_t = [[None] * 4 for _ in range(B)]
            # first chunk triplet first: it gates the first projection chains
            xq_t[0][0] = load_x(xq_pool, xq, 0, 0)
            xk_t[0][0] = load_x(xk_pool, xk, 0, 0)
            xv_t[0][0] = load_x(xv_pool, xv, 0, 0)
            nc.sync.dma_start(wq_t[:], wq[:])
            nc.sync.dma_start(wk_t[:], wk[:])
            nc.sync.dma_start(wv_t[:], wv[:])
            bq_t = persist.tile([DPC, 1], F32, tag="bq")
            bk_t = persist.tile([DPC, 1], F32, tag="bk")
            nc.sync.dma_start(bq_t[:], bq[:])
            nc.sync.dma_start(bk_t[:], bk[:])
            bv_bc = persist.tile([128, DPC], F32, tag="bvbc")
            nc.sync.dma_start(bv_bc[:], bv[:].partition_broadcast(128))
            mask_t = persist.tile([128, 128], BF16, tag="mask")
            nc.sync.dma_start(mask_t[:], mask[:])
            for b in range(B):
                for h in range(4):
                    if b == 0 and h == 0:
                        continue
                    xq_t[b][h] = load_x(xq_pool, xq, b, h)
                    xk_t[b][h] = load_x(xk_pool, xk, b, h)
                    xv_t[b][h] = load_x(xv_pool, xv, b, h)
                if b == 0:
                    nc.sync.dma_start(wo_t[:], wo[:])

            # ---- unit emitters ----
            def proj_chain_qk(b, sc, which):
                """one psum chain: Q or K projection for 512-token chunk sc"""
                scol = S * b
                xt_ = xq_t[b] if which == "q" else xk_t[b]
                wt_ = wq_t if which == "q" else wk_t
                bias = bq_t if which == "q" else bk_t
                dst = QT if which == "q" else KT
                ps = gen_ps.tile([128, 512], F32, tag="gen", name="gps")
                for j in range(8):
                    nc.tensor.matmul(
                        ps[:],
                        wt_[:, 128 * j : 128 * (j + 1)],
                        xt_[sc][:, 512 * j : 512 * (j + 1)],
                        start=(j == 0),
                        stop=(j == 7),
                    )
                nc.vector.tensor_scalar_add(
                    dst[:, scol + 512 * sc : scol + 512 * (sc + 1)], ps[:], bias[:]
                )

            def proj_chain_v(b, ss):
                """one psum chain: V projection for 128-token block ss"""
                h, jj = ss // 4, ss % 4
                ps = gen_ps.tile([128, 128], F32, tag="gen", name="gps")
                for j in range(8):
                    nc.tensor.matmul(
                        ps[:],
                        xv_t[b][h][:, 512 * j + 128 * jj : 512 * j + 128 * (jj + 1)],
                        wv_t[:, 128 * j : 128 * (j + 1)],
                        start=(j == 0),
                        stop=(j == 7),
                    )
                va = VA[NKT * b + ss]
                vb = VB[NKT * b + ss]
                nc.vector.tensor_add(va[:, 0:DK], ps[:, 0:DK], bv_bc[:, 0:DK])
                nc.vector.tensor_add(vb[:, 0:DK], ps[:, DK : 2 * DK], bv_bc[:, DK : 2 * DK])
                nc.vector.memset(va[:, DK : DK + 1], 1.0)
                nc.vector.memset(vb[:, DK : DK + 1], 1.0)

            def proj_unit_fillers(b, c):
                """chunk c of batch b -> 6 filler chains"""
                return [
                    lambda b=b, c=c: proj_chain_qk(b, c, "q"),
                    lambda b=b, c=c: proj_chain_qk(b, c, "k"),
                    lambda b=b, c=c: proj_chain_v(b, 4 * c + 0),
                    lambda b=b, c=c: proj_chain_v(b, 4 * c + 1),
                    lambda b=b, c=c: proj_chain_v(b, 4 * c + 2),
                    lambda b=b, c=c: proj_chain_v(b, 4 * c + 3),
                ]

            osb_t = {}

            def wo_chain(b, qb, ssub):
                """partial W_o for one 128-token slice (host sums partials)"""
                qsl0 = S * b + 512 * qb
                asl = slice(qsl0 + 128 * ssub, qsl0 + 128 * (ssub + 1))
                if ssub == 0:
                    osb_t[(b, qb)] = out_pool.tile([128, 4096], BF16, tag="osb", name="osb")
                osb = osb_t[(b, qb)]
                for nch in range(2):
                    ps = gen_ps.tile([128, 512], F32, tag="gen", name="gps")
                    nc.tensor.matmul(
                        ps[:],
                        AFT[:, asl],
                        wo_t[:, 512 * nch : 512 * (nch + 1)],
                        start=True,
                        stop=True,
                    )
                    nc.vector.tensor_copy(
                        osb[:, 1024 * ssub + 512 * nch : 1024 * ssub + 512 * (nch + 1)],
                        ps[:],
                    )
                ocol = (b * 4 + qb) * 4096
                if (b, qb) == (1, 3):
                    # final unit: store each 128-token slice as it completes
                    nc.gpsimd.dma_start(
                        out[:, ocol + 1024 * ssub : ocol + 1024 * (ssub + 1)],
                        osb[:, 1024 * ssub : 1024 * (ssub + 1)],
                    )
                elif ssub == 3:
                    nc.gpsimd.dma_start(out[:, ocol : ocol + 4096], osb[:])

            def wo_fillers(b, qb):
                return [lambda s=s: wo_chain(b, qb, s) for s in range(4)]

            avs_t = {}

            def attn_unit(b, qb, fillers):
                """scores/exp/mask/attnV for one q-block; fillers are emitted
                between kt iterations to keep the in-order PE queue fed
                during the exp-bound stretches"""
                scol = S * b
                qsl0 = scol + 512 * qb
                n_kt = 4 * qb + 4

                def emit_scores(kt):
                    t = kt - 4 * qb
                    lo = 128 * t if t >= 0 else 0
                    ksl = slice(scol + 128 * kt, scol + 128 * (kt + 1))
                    qsl = slice(qsl0 + lo, qsl0 + 512)
                    ps = score_ps.tile([128, 1024], F32, tag="sc", name="sc")
                    nc.tensor.matmul(
                        ps[:, lo:512], KT[0:64, ksl], QT[0:64, qsl],
                        start=True, stop=True,
                    )
                    nc.tensor.matmul(
                        ps[:, 512 + lo : 1024], KT[64:128, ksl], QT[64:128, qsl],
                        start=True, stop=True,
                    )
                    return ps

                av_a = av_ps.tile([DK + 1, 512], F32, tag="av_a", name="av_a")
                av_b = av_ps.tile([DK + 1, 512], F32, tag="av_b", name="av_b")
                fq = list(fillers)
                # spread fillers roughly evenly over the kt loop
                nf = len(fq)
                emit_at = set()
                if nf:
                    for i in range(nf):
                        emit_at.add(1 + (i * max(1, (n_kt - 1))) // nf)
                ps_cur = emit_scores(0)
                for kt in range(n_kt):
                    t = kt - 4 * qb
                    lo = 128 * t if t >= 0 else 0
                    et = exp_pool.tile([128, 1024], BF16, tag="et", name="et")
                    if t >= 0:
                        et3 = et[:].rearrange("p (h c) -> p h c", h=2)
                        ps3 = ps_cur[:].rearrange("p (h c) -> p h c", h=2)
                        nc.scalar.activation(
                            et3[:, :, lo:512], ps3[:, :, lo:512],
                            mybir.ActivationFunctionType.Exp, scale=0.125,
                        )
                    else:
                        nc.scalar.activation(
                            et[:], ps_cur[:], mybir.ActivationFunctionType.Exp,
                            scale=0.125,
                        )
                    if kt + 1 < n_kt:
                        ps_cur = emit_scores(kt + 1)
                    if t >= 0:
                        et3 = et[:].rearrange("p (h c) -> p h c", h=2)
                        nc.vector.tensor_mul(
                            et3[:, :, lo : lo + 128],
                            et3[:, :, lo : lo + 128],
                            mask_t[:].unsqueeze(1).broadcast_to((128, 2, 128)),
                        )
                    nc.tensor.matmul(
                        av_a[:, lo:512], VA[NKT * b + kt][:], et[:, lo:512],
                        start=(kt == 0), stop=(kt == n_kt - 1),
                    )
                    nc.tensor.matmul(
                        av_b[:, lo:512], VB[NKT * b + kt][:], et[:, 512 + lo : 1024],
                        start=(kt == 0), stop=(kt == n_kt - 1),
                    )
                    if kt in emit_at:
                        while fq and kt in emit_at:
                            fq.pop(0)()
                            break
                # evacuate psum promptly (frees attnV banks for next q-block)
                avs = small_pool.tile([DK + 1, 1024], F32, tag="avs", name="avs")
                nc.vector.tensor_copy(avs[:, 0:512], av_a[:])
                nc.vector.tensor_copy(avs[:, 512:1024], av_b[:])
                avs_t[(b, qb)] = avs
                for f in fq:
                    f()

            def fin_unit(b, qb):
                """normalize q-block qb into AFT (delayed one unit so the
                ACT-side reciprocal never blocks the exp queue)"""
                qsl = slice(S * b + 512 * qb, S * b + 512 * (qb + 1))
                avs = avs_t.pop((b, qb))
                # 1/den as exp(-ln(den)) on ScalarE, both heads in one pass
                lnd = small_pool.tile([1, 1024], F32, tag="scr", name="lnd")
                nc.scalar.activation(
                    lnd[:], avs[DK : DK + 1, :],
                    mybir.ActivationFunctionType.Ln,
                )
                rc = small_pool.tile([1, 1024], F32, tag="scr", name="rc")
                nc.scalar.activation(
                    rc[:], lnd[:], mybir.ActivationFunctionType.Exp,
                    scale=-1.0,
                )
                rbc = small_pool.tile([64, 1024], F32, tag="rbc", name="rbc")
                nc.gpsimd.partition_broadcast(rbc[:], rc[:])
                for hh in range(2):
                    nc.vector.tensor_mul(
                        AFT[64 * hh : 64 * (hh + 1), qsl],
                        avs[0:DK, 512 * hh : 512 * (hh + 1)],
                        rbc[:, 512 * hh : 512 * (hh + 1)],
                    )

            # ---- emission schedule (in-order engine queues: program order
            # is execution order, so software-pipeline across units) ----
            for f in proj_unit_fillers(0, 0):
                f()
            for f in proj_unit_fillers(0, 1):
                f()
            attn_unit(0, 0, [])
            for f in proj_unit_fillers(0, 2):
                f()
            fin_unit(0, 0)
            attn_unit(0, 1, wo_fillers(0, 0))
            for f in proj_unit_fillers(0, 3):
                f()
            fin_unit(0, 1)
            attn_unit(0, 2, wo_fillers(0, 1))
            fin_unit(0, 2)
            attn_unit(0, 3, proj_unit_fillers(1, 0) + wo_fillers(0, 2))
            fin_unit(0, 3)
            attn_unit(1, 0, wo_fillers(0, 3))
            for f in proj_unit_fillers(1, 1):
                f()
            fin_unit(1, 0)
            attn_unit(1, 1, proj_unit_fillers(1, 2)[:4] + wo_fillers(1, 0))
            for f in proj_unit_fillers(1, 2)[4:]:
                f()
            fin_unit(1, 1)
            attn_unit(1, 2, proj_unit_fillers(1, 3) + wo_fillers(1, 1))
            fin_unit(1, 2)
            attn_unit(1, 3, wo_fillers(1, 2))
            fin_unit(1, 3)
            for f in wo_fillers(1, 3):
                f()

    nc.finalize()
    return nc


def _prep_in_maps(q, k, v, W_q, b_q, W_k, b_k, W_v, b_v, W_o, b_o):
    def xT(x):  # [B,S,D] f32 -> [128, B*2*8*1024] bf16, chunked layout
        t = x.reshape(BS, D).T.astype(BF)  # [D, BS]
        # [j, p, b, sc, c] -> [p, b, sc, j, c]
        t = t.reshape(8, 128, B, 4, 512).transpose(1, 2, 3, 0, 4)
        return np.ascontiguousarray(t.reshape(128, B * 2 * 8 * 1024))

    def wpack(w):  # [D, 128] -> [128, 8*128]
        t = w.astype(BF).reshape(8, 128, DPC).transpose(1, 0, 2)
        return np.ascontiguousarray(t.reshape(128, 8 * DPC))

    xq_h, xk_h, xv_h = xT(q), xT(k), xT(v)

    i = np.arange(128)[:, None]
    j = np.arange(128)[None, :]
    mask_h = np.ascontiguousarray((i <= j).astype(BF))

    in_maps = []
    for c in range(N_CORES):
        csl = slice(DPC * c, DPC * (c + 1))
        in_maps.append(
            {
                "xq": xq_h,
                "xk": xk_h,
                "xv": xv_h,
                "wq": wpack(W_q[:, csl]),
                "wk": wpack(W_k[:, csl]),
                "wv": wpack(W_v[:, csl]),
                "wo": np.ascontiguousarray(W_o[csl, :].astype(BF)),
                "bq": np.ascontiguousarray(
                    b_q[csl].reshape(DPC, 1).astype(np.float32)
                ),
                "bk": np.ascontiguousarray(
                    b_k[csl].reshape(DPC, 1).astype(np.float32)
                ),
                "bv": np.ascontiguousarray(
                    b_v[csl].reshape(1, DPC).astype(np.float32)
                ),
                "mask": mask_h,
            }
        )
    return in_maps


def kernel(q, k, v, mask, W_q, b_q, W_k, b_k, W_v, b_v, W_o, b_o, **run_kwargs):
    q, k, v = (np.asarray(t, np.float32) for t in (q, k, v))
    b_o = np.asarray(b_o, np.float32)
    in_maps = _prep_in_maps(
        q, k, v,
        np.asarray(W_q, np.float32), np.asarray(b_q, np.float32),
        np.asarray(W_k, np.float32), np.asarray(b_k, np.float32),
        np.asarray(W_v, np.float32), np.asarray(b_v, np.float32),
        np.asarray(W_o, np.float32), b_o,
    )
    if "nc" not in _CACHED:
        _CACHED["nc"] = build_nc()
    res = run_bass_kernel_spmd(
        _CACHED["nc"], in_maps, core_ids=list(range(N_CORES)), **run_kwargs
    )
    _CACHED["last_result"] = res
    # host-side all-reduce of the 8 partial outputs (the unshard step)
    acc = None
    for c in range(N_CORES):
        part = np.asarray(res.results[c]["out"])  # [128, 32*1024] bf16
        part = part.reshape(128, B, 4, 4, 1024).transpose(1, 2, 3, 0, 4)
        part = part.reshape(BS, D).astype(np.float32)
        acc = part if acc is None else acc + part
    acc += b_o.reshape(1, D)
    return acc.reshape(B, S, D)


if __name__ == "__main__":
    build_nc()
    print("build ok")


# revision 17
# speedup vs baseline: 1.1574x; 1.0457x over previous
"""MultiHeadAttention forward on 8 Trainium2 NeuronCores.

Problem: B=2, S=2048, D_MODEL=1024, H=16 heads, d_k=64, causal mask.

Sharding: core c in [0,8) owns heads {2c, 2c+1} for BOTH batches.
 - Projections: each core computes Q^T,K^T ([128=2*d_k, B*S]) and V
   ([B*S, 2*65]) for its 2 heads (contraction over full d_model).
 - Attention in the "transposed scores" orientation: scoresT[kpos, q] =
   K^T.T @ Q^T per head (the two heads' 64-contraction matmuls run
   row-tiled/concurrently on the PE), exp on ScalarE (scale=1/8 folded
   in), causal handling is trimmed to the live column range of each
   diagonal tile plus a single [128,128] triangular mask multiply.
   attn_outT[dv, q] accumulates via matmuls with V tiles stationary; a
   ones-column in V yields softmax denominators in the same matmul.
 - Normalization: reciprocal_approx_fast of the sums row, gpsimd
   partition-broadcast, multiply into bf16 attn_flatT [128, B*S].
 - Output: NO collective. Each core multiplies its 128 attn_flatT rows
   by its 128 rows of W_o, producing a PARTIAL [B*S, D] output in bf16;
   the host sums the 8 partials and adds b_o (the all-reduce is the
   host-side unshard). W_o chunks + output stores pipeline per q-block,
   so no serial tail remains.

Matmuls run in bf16 (host-cast); accumulation is fp32 in PSUM.
"""

import sys

import numpy as np

sys.path.insert(0, "/opt/trn_rl_repo")

import ml_dtypes  # noqa: E402

import concourse.bacc as bacc  # noqa: E402
import concourse.mybir as mybir  # noqa: E402
import concourse.tile as tile  # noqa: E402
from concourse.bass_utils import run_bass_kernel_spmd  # noqa: E402

F32 = mybir.dt.float32
BF16 = mybir.dt.bfloat16
BF = ml_dtypes.bfloat16

B, S, D, H, DK = 2, 2048, 1024, 16, 64
N_CORES = 8
BS = B * S  # 4096
HPC = H // N_CORES  # heads per core = 2
DPC = HPC * DK  # d_model slice per core = 128

_CACHED = {}


class _Bacc(bacc.Bacc):
    """Bacc that pins the ACT function table to natural_log_exp_and_others.

    The default table pass picks the FIRST act_info.json set containing each
    activation: exp -> set 0, ln -> set 5, which makes every softmax-denominator
    reciprocal (ln then exp) swap the ACT table twice (~1.3us each) and
    serialize the exp queue. Removing exp/ln from every other set (list order
    and hence act_func_set ids unchanged) forces the one set that has both.
    """

    def insert_act_table_loads(self):
        import bass_rust as _bass_rust
        from concourse.hw_specs import get_activation_tables

        has_activation = any(
            isinstance(i, mybir.InstActivation)
            for blk in self.main_func.blocks
            for i in blk.instructions
        )
        if not has_activation:
            return
        A = mybir.ActivationFunctionType
        both = "natural_log_exp_and_others"
        tables = []
        for name, fns in get_activation_tables(self.m.arch).items():
            if name != both:
                fns = fns - {A.Exp, A.Ln}
            tables.append((name, fns))
        assert any(name == both and A.Exp in fns and A.Ln in fns for name, fns in tables)
        _bass_rust.insert_act_table_loads(self, tables)


def build_nc():
    nc = _Bacc(num_devices=N_CORES)

    # ---- I/O ----
    # x layout: [p, (b, h, j, c)] where element = x^T[128j+p, 2048b+1024h+c]
    xq = nc.dram_tensor("xq", [128, B * 2 * 8 * 1024], BF16, kind="ExternalInput")
    xk = nc.dram_tensor("xk", [128, B * 2 * 8 * 1024], BF16, kind="ExternalInput")
    xv = nc.dram_tensor("xv", [128, B * 2 * 8 * 1024], BF16, kind="ExternalInput")
    # w layout: [p, (j, c)] = W[128j+p, my_cols c]
    wq = nc.dram_tensor("wq", [128, 1024], BF16, kind="ExternalInput")
    wk = nc.dram_tensor("wk", [128, 1024], BF16, kind="ExternalInput")
    wv = nc.dram_tensor("wv", [128, 1024], BF16, kind="ExternalInput")
    wo = nc.dram_tensor("wo", [128, D], BF16, kind="ExternalInput")  # my 128 rows
    bq = nc.dram_tensor("bq", [DPC, 1], F32, kind="ExternalInput")
    bk = nc.dram_tensor("bk", [DPC, 1], F32, kind="ExternalInput")
    bv = nc.dram_tensor("bv", [1, DPC], F32, kind="ExternalInput")
    mask = nc.dram_tensor("mask", [128, 128], BF16, kind="ExternalInput")
    # out layout: [p, ((b*4+qb)*4 + ss)*1024 + c] = partial[2048b+512qb+128ss+p, c]
    out = nc.dram_tensor("out", [128, 32 * 1024], BF16, kind="ExternalOutput")

    NKT = S // 128  # kpos tiles per batch = 16
    NQB = S // 512  # q blocks per batch = 4

    with tile.TileContext(nc) as tc:
        with (
            tc.tile_pool(name="xtq", bufs=5) as xq_pool,
            tc.tile_pool(name="xtk", bufs=5) as xk_pool,
            tc.tile_pool(name="xtv", bufs=4) as xv_pool,
            tc.tile_pool(name="wtiles", bufs=1) as w_pool,
            tc.tile_pool(name="persist", bufs=1) as persist,
            tc.tile_pool(name="exp", bufs=4) as exp_pool,
            tc.tile_pool(name="outsb", bufs=2) as out_pool,
            tc.tile_pool(name="small", bufs=2) as small_pool,
            tc.tile_pool(name="gen_ps", bufs=2, space="PSUM") as gen_ps,
            tc.tile_pool(name="score_ps", bufs=2, space="PSUM") as score_ps,
            tc.tile_pool(name="av_ps", bufs=1, space="PSUM") as av_ps,
        ):
            # ---- persistent SBUF tensors ----
            QT = persist.tile([128, BS], BF16, tag="QT")  # rows: hA d 0-63, hB 64-127
            KT = persist.tile([128, BS], BF16, tag="KT")
            VA = [persist.tile([128, DK + 1], BF16, tag=f"VA{i}", name=f"VA{i}") for i in range(2 * NKT)]
            VB = [persist.tile([128, DK + 1], BF16, tag=f"VB{i}", name=f"VB{i}") for i in range(2 * NKT)]
            AFT = persist.tile([128, BS], BF16, tag="AFT")  # attn_flatT

            # ---- small loads first (tiny), then x chunks in use-order ----
            wq_t = persist.tile([128, 1024], BF16, tag="wqt")
            wk_t = persist.tile([128, 1024], BF16, tag="wkt")
            wv_t = persist.tile([128, 1024], BF16, tag="wvt")
            wo_t = persist.tile([128, D], BF16, tag="wot")
            # x chunk tiles, loaded per (batch, 512-token chunk): [128, 4096]
            def load_x(pool, src, b, h):
                t = pool.tile([128, 4096], BF16, tag="xc", name="xc")
                off = (b * 4 + h) * 4096
                nc.sync.dma_start(t[:], src[:, off : off + 4096])
                return t

            xq_t = [[None] * 4 for _ in range(B)]
            xk_t = [[None] * 4 for _ in range(B)]
            xv_t = [[None] * 4 for _ in range(B)]
            # first chunk triplet first: it gates the first projection chains
            xq_t[0][0] = load_x(xq_pool, xq, 0, 0)
            nc.sync.dma_start(wq_t[:], wq[:])
            nc.sync.dma_start(wk_t[:], wk[:])
            xk_t[0][0] = load_x(xk_pool, xk, 0, 0)
            nc.sync.dma_start(wv_t[:], wv[:])
            xv_t[0][0] = load_x(xv_pool, xv, 0, 0)
            bq_t = persist.tile([DPC, 1], F32, tag="bq")
            bk_t = persist.tile([DPC, 1], F32, tag="bk")
            nc.sync.dma_start(bq_t[:], bq[:])
            nc.sync.dma_start(bk_t[:], bk[:])
            bv_bc = persist.tile([128, DPC], F32, tag="bvbc")
            nc.sync.dma_start(bv_bc[:], bv[:].partition_broadcast(128))
            mask_t = persist.tile([128, 128], BF16, tag="mask")
            nc.sync.dma_start(mask_t[:], mask[:])
            for b in range(B):
                for h in range(4):
                    if b == 0 and h == 0:
                        continue
                    xq_t[b][h] = load_x(xq_pool, xq, b, h)
                    xk_t[b][h] = load_x(xk_pool, xk, b, h)
                    xv_t[b][h] = load_x(xv_pool, xv, b, h)
                if b == 0:
                    nc.sync.dma_start(wo_t[:], wo[:])

            # ---- unit emitters ----
            def proj_chain_qk(b, sc, which):
                """one psum chain: Q or K projection for 512-token chunk sc"""
                scol = S * b
                xt_ = xq_t[b] if which == "q" else xk_t[b]
                wt_ = wq_t if which == "q" else wk_t
                bias = bq_t if which == "q" else bk_t
                dst = QT if which == "q" else KT
                ps = gen_ps.tile([128, 512], F32, tag="gen", name="gps")
                for j in range(8):
                    nc.tensor.matmul(
                        ps[:],
                        wt_[:, 128 * j : 128 * (j + 1)],
                        xt_[sc][:, 512 * j : 512 * (j + 1)],
                        start=(j == 0),
                        stop=(j == 7),
                    )
                nc.vector.tensor_scalar_add(
                    dst[:, scol + 512 * sc : scol + 512 * (sc + 1)], ps[:], bias[:]
                )

            def proj_chain_v(b, ss):
                """one psum chain: V projection for 128-token block ss"""
                h, jj = ss // 4, ss % 4
                ps = gen_ps.tile([128, 128], F32, tag="gen", name="gps")
                for j in range(8):
                    nc.tensor.matmul(
                        ps[:],
                        xv_t[b][h][:, 512 * j + 128 * jj : 512 * j + 128 * (jj + 1)],
                        wv_t[:, 128 * j : 128 * (j + 1)],
                        start=(j == 0),
                        stop=(j == 7),
                    )
                va = VA[NKT * b + ss]
                vb = VB[NKT * b + ss]
                nc.vector.tensor_add(va[:, 0:DK], ps[:, 0:DK], bv_bc[:, 0:DK])
                nc.vector.tensor_add(vb[:, 0:DK], ps[:, DK : 2 * DK], bv_bc[:, DK : 2 * DK])
                nc.vector.memset(va[:, DK : DK + 1], 1.0)
                nc.vector.memset(vb[:, DK : DK + 1], 1.0)

            def proj_unit_fillers(b, c):
                """chunk c of batch b -> 6 filler chains"""
                return [
                    lambda b=b, c=c: proj_chain_qk(b, c, "q"),
                    lambda b=b, c=c: proj_chain_qk(b, c, "k"),
                    lambda b=b, c=c: proj_chain_v(b, 4 * c + 0),
                    lambda b=b, c=c: proj_chain_v(b, 4 * c + 1),
                    lambda b=b, c=c: proj_chain_v(b, 4 * c + 2),
                    lambda b=b, c=c: proj_chain_v(b, 4 * c + 3),
                ]

            osb_t = {}

            def wo_chain(b, qb, ssub):
                """partial W_o for one 128-token slice (host sums partials)"""
                qsl0 = S * b + 512 * qb
                asl = slice(qsl0 + 128 * ssub, qsl0 + 128 * (ssub + 1))
                if ssub == 0:
                    osb_t[(b, qb)] = out_pool.tile([128, 4096], BF16, tag="osb", name="osb")
                osb = osb_t[(b, qb)]
                for nch in range(2):
                    ps = gen_ps.tile([128, 512], F32, tag="gen", name="gps")
                    nc.tensor.matmul(
                        ps[:],
                        AFT[:, asl],
                        wo_t[:, 512 * nch : 512 * (nch + 1)],
                        start=True,
                        stop=True,
                    )
                    nc.vector.tensor_copy(
                        osb[:, 1024 * ssub + 512 * nch : 1024 * ssub + 512 * (nch + 1)],
                        ps[:],
                    )
                ocol = (b * 4 + qb) * 4096
                if (b, qb) == (1, 3):
                    # final unit: store each 128-token slice as it completes
                    nc.gpsimd.dma_start(
                        out[:, ocol + 1024 * ssub : ocol + 1024 * (ssub + 1)],
                        osb[:, 1024 * ssub : 1024 * (ssub + 1)],
                    )
                elif ssub == 3:
                    nc.gpsimd.dma_start(out[:, ocol : ocol + 4096], osb[:])

            def wo_fillers(b, qb):
                return [lambda s=s: wo_chain(b, qb, s) for s in range(4)]

            avs_t = {}

            def attn_unit(b, qb, fillers):
                """scores/exp/mask/attnV for one q-block; fillers are emitted
                between kt iterations to keep the in-order PE queue fed
                during the exp-bound stretches"""
                scol = S * b
                qsl0 = scol + 512 * qb
                n_kt = 4 * qb + 4

                def emit_scores(kt):
                    t = kt - 4 * qb
                    lo = 128 * t if t >= 0 else 0
                    ksl = slice(scol + 128 * kt, scol + 128 * (kt + 1))
                    qsl = slice(qsl0 + lo, qsl0 + 512)
                    ps = score_ps.tile([128, 1024], F32, tag="sc", name="sc")
                    nc.tensor.matmul(
                        ps[:, lo:512], KT[0:64, ksl], QT[0:64, qsl],
                        start=True, stop=True,
                    )
                    nc.tensor.matmul(
                        ps[:, 512 + lo : 1024], KT[64:128, ksl], QT[64:128, qsl],
                        start=True, stop=True,
                    )
                    return ps

                av_a = av_ps.tile([DK + 1, 512], F32, tag="av_a", name="av_a")
                av_b = av_ps.tile([DK + 1, 512], F32, tag="av_b", name="av_b")
                fq = list(fillers)
                ps_cur = emit_scores(0)
                # a couple of fillers right away: PE-queue work while the
                # first exp waits behind any fin ops on the ACT queue
                for _ in range(min(2, len(fq))):
                    fq.pop(0)()
                # spread the rest roughly evenly over the kt loop
                nf = len(fq)
                emit_at = set()
                if nf:
                    for i in range(nf):
                        emit_at.add(1 + (i * max(1, (n_kt - 1))) // nf)
                for kt in range(n_kt):
                    t = kt - 4 * qb
                    lo = 128 * t if t >= 0 else 0
                    et = exp_pool.tile([128, 1024], BF16, tag="et", name="et")
                    if t >= 0:
                        et3 = et[:].rearrange("p (h c) -> p h c", h=2)
                        ps3 = ps_cur[:].rearrange("p (h c) -> p h c", h=2)
                        nc.scalar.activation(
                            et3[:, :, lo:512], ps3[:, :, lo:512],
                            mybir.ActivationFunctionType.Exp, scale=0.125,
                        )
                    else:
                        nc.scalar.activation(
                            et[:], ps_cur[:], mybir.ActivationFunctionType.Exp,
                            scale=0.125,
                        )
                    if kt + 1 < n_kt:
                        ps_cur = emit_scores(kt + 1)
                    if t >= 0:
                        et3 = et[:].rearrange("p (h c) -> p h c", h=2)
                        nc.vector.tensor_mul(
                            et3[:, :, lo : lo + 128],
                            et3[:, :, lo : lo + 128],
                            mask_t[:].unsqueeze(1).broadcast_to((128, 2, 128)),
                        )
                    nc.tensor.matmul(
                        av_a[:, lo:512], VA[NKT * b + kt][:], et[:, lo:512],
                        start=(kt == 0), stop=(kt == n_kt - 1),
                    )
                    nc.tensor.matmul(
                        av_b[:, lo:512], VB[NKT * b + kt][:], et[:, 512 + lo : 1024],
                        start=(kt == 0), stop=(kt == n_kt - 1),
                    )
                    if kt in emit_at:
                        while fq and kt in emit_at:
                            fq.pop(0)()
                            break
                # evacuate psum promptly (frees attnV banks for next q-block)
                avs = small_pool.tile([DK + 1, 1024], F32, tag="avs", name="avs")
                nc.vector.tensor_copy(avs[:, 0:512], av_a[:])
                nc.vector.tensor_copy(avs[:, 512:1024], av_b[:])
                avs_t[(b, qb)] = avs
                for f in fq:
                    f()

            def fin_unit(b, qb):
                """normalize q-block qb into AFT (delayed one unit so the
                ACT-side reciprocal never blocks the exp queue)"""
                qsl = slice(S * b + 512 * qb, S * b + 512 * (qb + 1))
                avs = avs_t.pop((b, qb))
                # 1/den as exp(-ln(den)) on ScalarE, both heads in one pass
                lnd = small_pool.tile([1, 1024], F32, tag="scr", name="lnd")
                nc.scalar.activation(
                    lnd[:], avs[DK : DK + 1, :],
                    mybir.ActivationFunctionType.Ln,
                )
                rc = small_pool.tile([1, 1024], F32, tag="scr", name="rc")
                nc.scalar.activation(
                    rc[:], lnd[:], mybir.ActivationFunctionType.Exp,
                    scale=-1.0,
                )
                rbc = small_pool.tile([64, 1024], F32, tag="rbc", name="rbc")
                nc.gpsimd.partition_broadcast(rbc[:], rc[:])
                for hh in range(2):
                    nc.vector.tensor_mul(
                        AFT[64 * hh : 64 * (hh + 1), qsl],
                        avs[0:DK, 512 * hh : 512 * (hh + 1)],
                        rbc[:, 512 * hh : 512 * (hh + 1)],
                    )

            # ---- emission schedule (in-order engine queues: program order
            # is execution order, so software-pipeline across units) ----
            for f in proj_unit_fillers(0, 0):
                f()
            for f in proj_unit_fillers(0, 1):
                f()
            attn_unit(0, 0, [])
            for f in proj_unit_fillers(0, 2):
                f()
            fin_unit(0, 0)
            attn_unit(0, 1, wo_fillers(0, 0))
            for f in proj_unit_fillers(0, 3):
                f()
            fin_unit(0, 1)
            attn_unit(0, 2, wo_fillers(0, 1))
            fin_unit(0, 2)
            attn_unit(0, 3, proj_unit_fillers(1, 0) + wo_fillers(0, 2))
            attn_unit(1, 0, proj_unit_fillers(1, 1)[:4])
            fin_unit(0, 3)
            for f in proj_unit_fillers(1, 1)[4:]:
                f()
            fin_unit(1, 0)
            attn_unit(1, 1, wo_fillers(0, 3) + wo_fillers(1, 0))
            for f in proj_unit_fillers(1, 2):
                f()
            fin_unit(1, 1)
            attn_unit(1, 2, proj_unit_fillers(1, 3))
            fin_unit(1, 2)
            attn_unit(1, 3, wo_fillers(1, 1) + wo_fillers(1, 2))
            fin_unit(1, 3)
            for f in wo_fillers(1, 3):
                f()

    nc.finalize()
    return nc


def _prep_in_maps(q, k, v, W_q, b_q, W_k, b_k, W_v, b_v, W_o, b_o):
    def xT(x):  # [B,S,D] f32 -> [128, B*2*8*1024] bf16, chunked layout
        t = x.reshape(BS, D).T.astype(BF)  # [D, BS]
        # [j, p, b, sc, c] -> [p, b, sc, j, c]
        t = t.reshape(8, 128, B, 4, 512).transpose(1, 2, 3, 0, 4)
        return np.ascontiguousarray(t.reshape(128, B * 2 * 8 * 1024))

    def wpack(w):  # [D, 128] -> [128, 8*128]
        t = w.astype(BF).reshape(8, 128, DPC).transpose(1, 0, 2)
        return np.ascontiguousarray(t.reshape(128, 8 * DPC))

    xq_h, xk_h, xv_h = xT(q), xT(k), xT(v)

    i = np.arange(128)[:, None]
    j = np.arange(128)[None, :]
    mask_h = np.ascontiguousarray((i <= j).astype(BF))

    in_maps = []
    for c in range(N_CORES):
        csl = slice(DPC * c, DPC * (c + 1))
        in_maps.append(
            {
                "xq": xq_h,
                "xk": xk_h,
                "xv": xv_h,
                "wq": wpack(W_q[:, csl]),
                "wk": wpack(W_k[:, csl]),
                "wv": wpack(W_v[:, csl]),
                "wo": np.ascontiguousarray(W_o[csl, :].astype(BF)),
                "bq": np.ascontiguousarray(
                    b_q[csl].reshape(DPC, 1).astype(np.float32)
                ),
                "bk": np.ascontiguousarray(
                    b_k[csl].reshape(DPC, 1).astype(np.float32)
                ),
                "bv": np.ascontiguousarray(
                    b_v[csl].reshape(1, DPC).astype(np.float32)
                ),
                "mask": mask_h,
            }
        )
    return in_maps


def kernel(q, k, v, mask, W_q, b_q, W_k, b_k, W_v, b_v, W_o, b_o, **run_kwargs):
    q, k, v = (np.asarray(t, np.float32) for t in (q, k, v))
    b_o = np.asarray(b_o, np.float32)
    in_maps = _prep_in_maps(
        q, k, v,
        np.asarray(W_q, np.float32), np.asarray(b_q, np.float32),
        np.asarray(W_k, np.float32), np.asarray(b_k, np.float32),
        np.asarray(W_v, np.float32), np.asarray(b_v, np.float32),
        np.asarray(W_o, np.float32), b_o,
    )
    if "nc" not in _CACHED:
        _CACHED["nc"] = build_nc()
    res = run_bass_kernel_spmd(
        _CACHED["nc"], in_maps, core_ids=list(range(N_CORES)), **run_kwargs
    )
    _CACHED["last_result"] = res
    # host-side all-reduce of the 8 partial outputs (the unshard step)
    acc = None
    for c in range(N_CORES):
        part = np.asarray(res.results[c]["out"])  # [128, 32*1024] bf16
        part = part.reshape(128, B, 4, 4, 1024).transpose(1, 2, 3, 0, 4)
        part = part.reshape(BS, D).astype(np.float32)
        acc = part if acc is None else acc + part
    acc += b_o.reshape(1, D)
    return acc.reshape(B, S, D)


if __name__ == "__main__":
    build_nc()
    print("build ok")
